# revision 10
# baseline (speedup 1.0000x reference)
"""Trainium2 Bass kernel for the masked-attention block (nn_MAB_61607010894006).

Sharding: data-parallel over batch B=8 across 8 NeuronCores (one batch row
per core, weights replicated, no collectives).

v4 design (vs 162us v2 / 177us v3):
  - ACT (scalar) engine is the fundamental bottleneck: ~5.2M softmax exps
    at 1 elem/cycle/lane @1.2GHz.  Everything else is arranged around
    keeping the ACT exp stream dense and the PE warm (HAM clock gate).
  - Softmax exps merged: scores per (head, ktile) land in a 2-bank
    [128,1024] PSUM tile (2 row-tiled matmuls per chunk), ONE exp each.
  - Software pipeline: window t emits scores(t)+exps(t) interleaved with
    proj(t+1) (early, so the next window's scores aren't gated on the
    proj->cast latency) and A@V(t-1) + normalize(t-1) as PE/DVE filler.
  - Softmax denominators: ride A@V as the 65th v row; po[0:65] is drained
    to SBUF bf16 immediately (frees the PSUM bank); the denom row is
    DMA-hopped (2KB) to partitions 0/32 of a per-pair gather tile so ONE
    Ln + ONE Exp [33,1024] serves a whole head pair (ACT lanes in
    parallel instead of 1-lane [1,512] ops).
  - LN0 sums accumulate in-window (PE filler); LN1/FC epilogue balances
    DVE (relu, applies) vs ACT (squares, rr/rm casts).
"""

import sys

sys.path.insert(0, "/opt/trn_rl_repo")

import numpy as np
import ml_dtypes

import concourse.bass as bass
import concourse.mybir as mybir
import concourse.tile as tile
from concourse.bass_utils import run_bass_kernel_spmd


F32 = mybir.dt.float32
BF16 = mybir.dt.bfloat16
AF = mybir.ActivationFunctionType

B, NQ, NK, D, H, DH = 8, 1024, 1024, 512, 8, 64
EPS = 1e-5
N_CORES = 8


def _split_multi_waits(nc):
    """This toolchain's walrus allows ONE sem wait per TPB instruction; Tile
    can emit several (kernel-tail drain). Hoist extras onto preceding
    single-wait NOPs on the same engine stream (equivalent: in-order issue).
    """
    multi_update = []
    for fn in nc.m.functions:
        for bb in fn.blocks:
            insts = bb.instructions
            new = []
            changed = False
            for inst in insts:
                si = inst.sync_info
                if si is not None and si.on_wait and len(si.on_wait) > 1:
                    waits = list(si.on_wait)
                    for w in waits[:-1]:
                        nop = mybir.InstNoOp(
                            name=f"I-wsplit-{nc.next_id()}", engine=inst.engine
                        )
                        nop.sync_info = mybir.SyncInfo(on_wait=[w], on_update=[])
                        new.append(nop)
                    inst.sync_info = mybir.SyncInfo(
                        on_wait=[waits[-1]], on_update=list(si.on_update)
                    )
                    changed = True
                if si is not None and si.on_update and len(si.on_update) > 1:
                    multi_update.append(inst.name)
                new.append(inst)
            if changed:
                bb.instructions = new
    if multi_update:
        raise RuntimeError(f">1 sem update unsupported: {multi_update[:10]}")


def build_nc(kt_tiles=5, affine=False):
    NKP = kt_tiles * 128  # compacted+padded key/value token count
    nc = bass.Bass()

    qt_d = nc.dram_tensor("qt", [D, NQ], BF16, kind="ExternalInput")
    kt_d = nc.dram_tensor("kt", [D + 1, NKP], BF16, kind="ExternalInput")  # +ind
    wq_d = nc.dram_tensor("wq", [D, D], BF16, kind="ExternalInput")
    wk_d = nc.dram_tensor("wk", [D, D], BF16, kind="ExternalInput")
    wv_d = nc.dram_tensor("wv", [D + 1, D], BF16, kind="ExternalInput")  # +bv row
    wo_d = nc.dram_tensor("wo", [D, D], BF16, kind="ExternalInput")
    bq_d = nc.dram_tensor("bq", [128, 4], F32, kind="ExternalInput")
    bk_d = nc.dram_tensor("bk", [128, 4], F32, kind="ExternalInput")
    bo_d = nc.dram_tensor("bo", [128, 4], F32, kind="ExternalInput")
    ind_d = nc.dram_tensor("ind", [128, kt_tiles], BF16, kind="ExternalInput")
    cr_d = nc.dram_tensor("cr", [33, 128], BF16, kind="ExternalInput")  # ones @0,32
    gb_d = nc.dram_tensor("gb", [128, 16], F32, kind="ExternalInput")  # percol
    cn_d = nc.dram_tensor("cn", [128, 1], BF16, kind="ExternalInput")  # 1/512
    out_d = nc.dram_tensor("out", [D, NQ], BF16, kind="ExternalOutput")

    mult, add = mybir.AluOpType.mult, mybir.AluOpType.add
    amax = mybir.AluOpType.max

    def mm(out, lhsT, rhs, **kw):
        nc.tensor.matmul(out, lhsT, rhs, **kw)

    with tile.TileContext(nc) as tc:
        with (
            tc.tile_pool(name="wp", bufs=1) as wp,
            tc.tile_pool(name="ap", bufs=1) as ap,
            tc.tile_pool(name="sm", bufs=2) as sm,
            tc.tile_pool(name="pp", bufs=1, space="PSUM") as pp,
        ):
            # PSUM: 8 banks.  sx = 2x [128,1024] (4) proj/scores/FC;
            # po = 2x [128,512] (2) A@V + pb + LN broadcasts;
            # spb = 2x [65,512] (2) LN stat sums (live across windows).
            def sx_tile(name):
                return pp.tile([128, 1024], F32, name=name, tag="sx", bufs=2)

            def po_tile(name):
                return pp.tile([128, 512], F32, name=name, tag="po", bufs=2)

            def spb_tile(name):
                return pp.tile([65, 512], F32, name=name, tag="spb", bufs=2)

            # ---- small constants first (cheap DMAs) ---------------------------
            bq_sb = wp.tile([128, 4], F32, name="bq_sb")
            bk_sb = wp.tile([128, 4], F32, name="bk_sb")
            bo_sb = wp.tile([128, 4], F32, name="bo_sb")
            ind_sb = wp.tile([128, kt_tiles], BF16, name="ind_sb")
            cr_sb = wp.tile([33, 128], BF16, name="cr_sb")
            gb_sb = wp.tile([128, 16], F32, name="gb_sb")
            cn_sb = wp.tile([128, 1], BF16, name="cn_sb")
            nc.sync.dma_start(bq_sb[:], bq_d[:])
            nc.sync.dma_start(bk_sb[:], bk_d[:])
            nc.sync.dma_start(bo_sb[:], bo_d[:])
            nc.sync.dma_start(ind_sb[:], ind_d[:])
            nc.sync.dma_start(cr_sb[:], cr_d[:])
            nc.sync.dma_start(gb_sb[:], gb_d[:])
            nc.sync.dma_start(cn_sb[:], cn_d[:])
            eps_sb = wp.tile([1, 1], F32, name="eps_sb")
            nc.vector.memset(eps_sb[:], EPS)
            sum_lhs = cn_sb[:, 0:1]                    # [128,1] bf16 = 1/512
            ones_row = cr_sb[0:1, :]                   # [1,128] bf16 lhsT

            # ---- PE warmup while DMAs stream ---------------------------------
            wu_sb = wp.tile([128, 128], BF16, name="wu_sb")
            nc.vector.memset(wu_sb[:], 0.001)
            wu_ps = sx_tile("wu_ps")
            for i in range(44):
                mm(wu_ps[:, 0:128], wu_sb[:], wu_sb[:],
                   start=(i == 0), stop=(i == 43))
            wu_out = wp.tile([1, 1], F32, name="wu_out")
            nc.vector.tensor_copy(wu_out[:], wu_ps[0:1, 0:1])

            # ---- inputs: kt+wv first (v/k proj start earliest), spread over
            # the 3 DMA-capable queues (sync/SP, scalar/ACT, gpsimd) ----------
            wq_sb = wp.tile([128, 4 * D], BF16, name="wq_sb")
            wk_sb = wp.tile([128, 4 * D], BF16, name="wk_sb")
            wv_sb = wp.tile([128, 4 * D], BF16, name="wv_sb")
            wv1_sb = wp.tile([1, D], BF16, name="wv1_sb")
            wo_sb = wp.tile([128, 4 * D], BF16, name="wo_sb")
            kt_sb = wp.tile([128, 4 * NKP], BF16, name="kt_sb")
            kt1_sb = wp.tile([1, NKP], BF16, name="kt1_sb")
            qt_sb = wp.tile([128, 4 * NQ], BF16, name="qt_sb")
            QS = [nc.sync, nc.scalar, nc.gpsimd]
            qi = 0

            def dma(dst, src):
                nonlocal qi
                QS[qi % 3].dma_start(dst, src)
                qi += 1

            for t in range(4):
                dma(kt_sb[:, t * NKP : (t + 1) * NKP],
                    kt_d[t * 128 : (t + 1) * 128, :])
            nc.sync.dma_start(kt1_sb[:, :], kt_d[D : D + 1, :])
            for t in range(4):
                dma(wv_sb[:, t * D : (t + 1) * D],
                    wv_d[t * 128 : (t + 1) * 128, :])
            nc.scalar.dma_start(wv1_sb[:, :], wv_d[D : D + 1, :])
            for t in range(4):
                dma(wk_sb[:, t * D : (t + 1) * D],
                    wk_d[t * 128 : (t + 1) * 128, :])
            for t in range(4):
                dma(qt_sb[:, t * NQ : (t + 1) * NQ],
                    qt_d[t * 128 : (t + 1) * 128, :])
            for t in range(4):
                dma(wq_sb[:, t * D : (t + 1) * D],
                    wq_d[t * 128 : (t + 1) * 128, :])
            for t in range(4):
                dma(wo_sb[:, t * D : (t + 1) * D],
                    wo_d[t * 128 : (t + 1) * 128, :])

            # ---- persistent activations --------------------------------------
            q_bf = ap.tile([128, 4 * NQ], BF16, name="q_bf")
            k_bf = ap.tile([128, 4 * NKP], BF16, name="k_bf")
            v_sb = ap.tile([128, kt_tiles * (8 * 65)], BF16, name="v_sb")
            v_ones = v_sb.rearrange("p (v h x) -> p v h x", v=kt_tiles, h=8)[
                :, :, :, 64
            ]
            nc.vector.tensor_copy(
                v_ones,
                ind_sb.rearrange("p (v a) -> p v a", a=1)
                .broadcast_to([128, kt_tiles, 8]),
            )

            # ---- v projection (token-major, +bias via ind row) ---------------
            for vt in range(kt_tiles):
                ps_v = sx_tile(f"ps_v{vt}")
                for kc in range(4):
                    mm(
                        ps_v[:, 0:512],
                        kt_sb[:, kc * NKP + vt * 128 : kc * NKP + (vt + 1) * 128],
                        wv_sb[:, kc * D : (kc + 1) * D],
                        start=(kc == 0),
                        stop=False,
                    )
                mm(
                    ps_v[:, 0:512],
                    kt1_sb[0:1, vt * 128 : (vt + 1) * 128],
                    wv1_sb[0:1, :],
                    start=False,
                    stop=True,
                )
                v_dst = v_sb[:, vt * 520 : (vt + 1) * 520].rearrange(
                    "p (h x) -> p h x", h=8
                )[:, :, 0:64]
                nc.scalar.copy(v_dst, ps_v[:, 0:512].rearrange("p (h x) -> p h x", h=8))

            # ---- projections for one t-block ---------------------------------
            def proj_t(t):
                ps_k = sx_tile(f"ps_k{t}")
                kchunks = [(0, min(NKP, 512))] + (
                    [(512, NKP - 512)] if NKP > 512 else []
                )
                for kc in range(4):
                    for cs, cw in kchunks:
                        mm(
                            ps_k[:, cs : cs + cw],
                            wk_sb[:, kc * D + t * 128 : kc * D + (t + 1) * 128],
                            kt_sb[:, kc * NKP + cs : kc * NKP + cs + cw],
                            start=(kc == 0),
                            stop=(kc == 3),
                        )
                nc.vector.tensor_scalar_add(
                    k_bf[:, t * NKP : (t + 1) * NKP],
                    ps_k[:, 0:NKP],
                    bk_sb[:, t : t + 1],
                )
                ps_q = sx_tile(f"ps_q{t}")
                for kc in range(4):
                    for c in range(2):
                        mm(
                            ps_q[:, c * 512 : c * 512 + 512],
                            wq_sb[:, kc * D + t * 128 : kc * D + (t + 1) * 128],
                            qt_sb[:, kc * NQ + c * 512 : kc * NQ + c * 512 + 512],
                            start=(kc == 0),
                            stop=(kc == 3),
                        )
                nc.vector.tensor_scalar_add(
                    q_bf[:, t * NQ : (t + 1) * NQ],
                    ps_q[:],
                    bq_sb[:, t : t + 1],
                )

            o_bf = ap.tile([128, 4 * NQ], BF16, name="o_bf")
            at_tiles = {}  # t -> {sub: [tiles]}
            av_tiles = {}  # h -> av_sb [65, NQ]
            dg_tiles = {}  # pair -> [33, NQ]
            sqt_tiles = []
            sts0 = [spb_tile(f"stl0{c}") for c in range(2)]

            def scores_i(t, i):
                """Scores + exp for ktile i of head pair (2t, 2t+1)."""
                pss = {}
                for sub in range(2):
                    pss[sub] = sx_tile(f"s{t}_{i}_{sub}")
                for c in range(2):
                    for sub in range(2):
                        rh = sub * 64
                        mm(
                            pss[sub][:, c * 512 : c * 512 + 512],
                            k_bf[rh : rh + 64,
                                 t * NKP + i * 128 : t * NKP + (i + 1) * 128],
                            q_bf[rh : rh + 64,
                                 t * NQ + c * 512 : t * NQ + c * 512 + 512],
                            start=True,
                            stop=True,
                        )
                for sub in range(2):
                    nc.scalar.activation(
                        at_tiles[t][sub][i][:], pss[sub][:], AF.Exp, scale=0.125
                    )

            def av_head(t, sub):
                """A@V for head 2t+sub; drain AV rows + denom row to SBUF."""
                h = 2 * t + sub
                av_sb = sm.tile([65, NQ], BF16, name=f"av{h}", tag="avs", bufs=4)
                av_tiles[h] = av_sb
                if sub == 0:
                    dg_tiles[t] = sm.tile([33, NQ], BF16, name=f"dg{t}",
                                          tag="dg", bufs=2)
                for c in range(2):
                    po = po_tile(f"po{h}_{c}")
                    for i in range(kt_tiles):
                        mm(
                            po[0:65, :],
                            v_sb[:, i * 520 + h * 65 : i * 520 + (h + 1) * 65],
                            at_tiles[t][sub][i][:, c * 512 : (c + 1) * 512],
                            start=(i == 0),
                            stop=(i == kt_tiles - 1),
                        )
                    nc.vector.tensor_copy(
                        av_sb[:, c * 512 : c * 512 + 512], po[0:65, :]
                    )
                # hop the denom row (2KB) to partition 32*sub of the pair tile
                nc.sync.dma_start(
                    dg_tiles[t][32 * sub : 32 * sub + 1, :], av_sb[64:65, :]
                )

            def norm_pair(t):
                """1/denom for pair t (ONE Ln + ONE Exp over [33,1024]),
                broadcast, normalize, residual add, sqt, LN0 sums."""
                tsl = slice(t * NQ, (t + 1) * NQ)
                dg = dg_tiles[t]
                lnd = sm.tile([33, NQ], F32, name=f"lnd{t}", tag="lnd", bufs=2)
                nc.scalar.activation(lnd[:], dg[:], AF.Ln)
                rinv = sm.tile([33, NQ], BF16, name=f"ri{t}", tag="ri", bufs=2)
                nc.scalar.activation(rinv[:], lnd[:], AF.Exp, scale=-1.0)
                for sub in range(2):
                    h = 2 * t + sub
                    rp = 32 * sub
                    avn = sm.tile([64, NQ], BF16, name=f"avn{h}", tag="avn",
                                  bufs=2)
                    for c in range(2):
                        csl = slice(c * 512, c * 512 + 512)
                        pb = po_tile(f"pb{h}_{c}")
                        mm(pb[0:64, :], cr_sb[rp : rp + 1, 0:64],
                           rinv[rp : rp + 1, csl], start=True, stop=True)
                        nc.vector.tensor_mul(
                            avn[:, csl], av_tiles[h][0:64, csl], pb[0:64, :]
                        )
                    if sub == 0:
                        nc.vector.tensor_add(
                            o_bf[0:64, tsl], avn[:], q_bf[0:64, tsl]
                        )
                    else:
                        av2 = sm.tile([128, NQ], BF16, name=f"av2_{h}",
                                      tag="av2", bufs=2)
                        nc.gpsimd.dma_start(av2[64:128, :], avn[:])
                        nc.vector.tensor_add(
                            o_bf[64:128, tsl], av2[64:128, :], q_bf[64:128, tsl]
                        )
                sqt = sm.tile([128, NQ], BF16, name=f"sqt{t}", tag="sqt", bufs=4)
                sqt_tiles.append(sqt)
                nc.vector.tensor_mul(sqt[:], o_bf[:, tsl], o_bf[:, tsl])
                # LN0 partial sums for block t (PE filler, col-tiled rows 0/64)
                for c in range(2):
                    csl = slice(t * NQ + c * 512, t * NQ + c * 512 + 512)
                    mm(sts0[c][0:1, :], sum_lhs, o_bf[:, csl],
                       start=(t == 0), stop=(t == 3))
                    mm(sts0[c][64:65, :], sum_lhs,
                       sqt[:, c * 512 : c * 512 + 512],
                       start=(t == 0), stop=(t == 3))

            # ---- pipelined main loop -----------------------------------------
            proj_t(0)
            for t in range(4):
                at_tiles[t] = {
                    sub: [
                        ap.tile([128, NQ], BF16, name=f"at{t}_{i}_{sub}",
                                tag="at", bufs=20)
                        for i in range(kt_tiles)
                    ]
                    for sub in range(2)
                }
                scores_i(t, 0)
                if kt_tiles > 1:
                    scores_i(t, 1)
                if t < 3:
                    proj_t(t + 1)
                for i in range(2, kt_tiles):
                    scores_i(t, i)
                    if t > 0:
                        if i == 2:
                            av_head(t - 1, 0)
                        elif i == 3:
                            av_head(t - 1, 1)
                if t > 0:
                    if kt_tiles <= 2:
                        av_head(t - 1, 0)
                    if kt_tiles <= 3:
                        av_head(t - 1, 1)
                    norm_pair(t - 1)
            av_head(3, 0)
            av_head(3, 1)
            norm_pair(3)

            # ---- LN stats + broadcast helpers --------------------------------
            def ln_stats_and_reps(sts, tag):
                """sts: per-c [65,512] f32 PSUM (mean row 0, meansq row 64).
                Returns (rr, rm): [128, NQ] bf16 broadcasts of rstd, mu*rstd."""
                var = sm.tile([1, NQ], F32, name=f"var{tag}", tag="var", bufs=2)
                mu = sm.tile([1, NQ], F32, name=f"mu{tag}", tag="mu", bufs=2)
                for c in range(2):
                    csl = slice(c * 512, (c + 1) * 512)
                    nc.scalar.activation(var[:, csl], sts[c][0:1, :], AF.Square)
                    nc.vector.tensor_sub(var[:, csl], sts[c][64:65, :],
                                         var[:, csl])
                    nc.vector.tensor_copy(mu[:, csl], sts[c][0:1, :])
                rstd = sm.tile([1, NQ], BF16, name=f"rs{tag}", tag="rs", bufs=2)
                murm = sm.tile([1, NQ], BF16, name=f"mm{tag}", tag="mm2", bufs=2)
                rr = sm.tile([128, NQ], BF16, name=f"rrb{tag}", tag="rrb", bufs=2)
                rm = sm.tile([128, NQ], BF16, name=f"rmb{tag}", tag="rmb", bufs=2)
                for c in range(2):
                    csl = slice(c * 512, (c + 1) * 512)
                    nc.scalar.activation(var[:, csl], var[:, csl], AF.Ln,
                                         bias=eps_sb[0:1, 0:1])
                    nc.scalar.activation(rstd[:, csl], var[:, csl], AF.Exp,
                                         scale=-0.5)
                    nc.vector.tensor_mul(murm[:, csl], mu[:, csl],
                                         rstd[:, csl])
                    rr_ps = po_tile(f"rr{tag}{c}")
                    mm(rr_ps[:, :], ones_row, rstd[:, csl],
                       start=True, stop=True)
                    nc.scalar.copy(rr[:, csl], rr_ps[:, :])
                    rm_ps = po_tile(f"rm{tag}{c}")
                    mm(rm_ps[:, :], ones_row, murm[:, csl],
                       start=True, stop=True)
                    nc.scalar.copy(rm[:, csl], rm_ps[:, :])
                return rr, rm

            # ---- LN0 ----------------------------------------------------------
            ot0 = ap.tile([128, 4 * NQ], BF16, name="ot0")
            rr0, rm0 = ln_stats_and_reps(sts0, "l0")
            for t in range(4):
                tsl = slice(t * NQ, (t + 1) * NQ)
                nc.vector.tensor_mul(ot0[:, tsl], o_bf[:, tsl], rr0[:])
                nc.vector.tensor_sub(ot0[:, tsl], ot0[:, tsl], rm0[:])
                if affine:
                    for c in range(2):
                        sl = slice(t * NQ + c * 512, t * NQ + c * 512 + 512)
                        nc.vector.tensor_scalar(
                            ot0[:, sl], ot0[:, sl],
                            gb_sb[:, 0 + t : 0 + t + 1],
                            gb_sb[:, 4 + t : 4 + t + 1],
                            mult, add,
                        )

            # ---- FC + relu(DVE) + residual; squares on ACT --------------------
            o1 = ap.tile([128, 4 * NQ], BF16, name="o1")
            sq1_tiles = [
                sm.tile([128, NQ], BF16, name=f"sq1t{ot}", tag="sqt", bufs=4)
                for ot in range(4)
            ]
            sts1 = [spb_tile(f"stl1{c}") for c in range(2)]
            for c in range(2):
                for ot in range(4):
                    ps_f = sx_tile(f"psf{ot}_{c}")
                    for ft in range(4):
                        mm(
                            ps_f[:, 0:512],
                            wo_sb[:, ft * D + ot * 128 : ft * D + (ot + 1) * 128],
                            ot0[:, ft * NQ + c * 512 : ft * NQ + c * 512 + 512],
                            start=(ft == 0),
                            stop=(ft == 3),
                        )
                    csl = slice(ot * NQ + c * 512, ot * NQ + c * 512 + 512)
                    rl = sm.tile([128, 512], BF16, name=f"rl{ot}{c}", tag="rl",
                                 bufs=2)
                    nc.vector.tensor_scalar(
                        rl[:], ps_f[:, 0:512], bo_sb[:, ot : ot + 1], 0.0,
                        add, amax,
                    )
                    nc.vector.tensor_add(o1[:, csl], ot0[:, csl], rl[:])
                    sq1t = sq1_tiles[ot]
                    nc.scalar.activation(
                        sq1t[:, c * 512 : c * 512 + 512], o1[:, csl], AF.Square
                    )
                    mm(sts1[c][0:1, :], sum_lhs, o1[:, csl],
                       start=(ot == 0), stop=(ot == 3))
                    mm(sts1[c][64:65, :], sum_lhs,
                       sq1t[:, c * 512 : c * 512 + 512],
                       start=(ot == 0), stop=(ot == 3))

            # ---- LN1 -> out ---------------------------------------------------
            otout = ap.tile([128, 4 * NQ], BF16, name="otout")
            rr1, rm1 = ln_stats_and_reps(sts1, "l1")
            for t in range(4):
                for c in range(2):
                    sl = slice(t * NQ + c * 512, t * NQ + c * 512 + 512)
                    rsl = slice(c * 512, c * 512 + 512)
                    tmp = sm.tile([128, 512], BF16, name=f"tmp{t}{c}", tag="rl",
                                  bufs=2)
                    nc.vector.tensor_mul(tmp[:], o1[:, sl], rr1[:, rsl])
                    if affine:
                        nc.vector.tensor_sub(tmp[:], tmp[:], rm1[:, rsl])
                        nc.vector.tensor_scalar(
                            otout[:, sl], tmp[:],
                            gb_sb[:, 8 + t : 8 + t + 1],
                            gb_sb[:, 12 + t : 12 + t + 1],
                            mult, add,
                        )
                    else:
                        nc.vector.tensor_sub(otout[:, sl], tmp[:], rm1[:, rsl])
                    (nc.sync if c == 0 else nc.gpsimd).dma_start(
                        out_d[t * 128 : (t + 1) * 128, c * 512 : c * 512 + 512],
                        otout[:, sl],
                    )

    _split_multi_waits(nc)
    return nc


_nc_cache = {}


def _get_nc(kt_tiles=5, affine=False):
    key = (kt_tiles, affine)
    if key not in _nc_cache:
        _nc_cache[key] = build_nc(kt_tiles, affine)
    return _nc_cache[key]


def _kt_tiles_for(mask):
    n = int(max(int((mask[b] != 0).sum()) for b in range(mask.shape[0])))
    return max(1, (n + 127) // 128)


def _is_affine(g0, b0, g1, b1):
    return not (
        np.all(np.asarray(g0) == 1.0)
        and np.all(np.asarray(b0) == 0.0)
        and np.all(np.asarray(g1) == 1.0)
        and np.all(np.asarray(b1) == 0.0)
    )


def prep_inputs(Q, K, mask, Wq, bq, Wk, bk, Wv, bv, Wo, bo, g0, b0, g1, b1,
                kt_tiles=None):
    f32, bf = np.float32, ml_dtypes.bfloat16
    if kt_tiles is None:
        kt_tiles = _kt_tiles_for(np.asarray(mask))
    nkp = kt_tiles * 128

    def percol(v):  # [512] feature vector -> [128, 4] per-partition layout
        return np.ascontiguousarray(np.asarray(v, f32).reshape(4, 128).T)

    wv_h = np.ascontiguousarray(
        np.vstack([np.asarray(Wv, f32), np.asarray(bv, f32)[None, :]])
    ).astype(bf)
    cr = np.zeros((33, 128), f32)
    cr[0, :] = 1.0
    cr[32, :] = 1.0
    cr = cr.astype(bf)
    gb = np.concatenate(
        [percol(g0), percol(b0), percol(g1), percol(b1)], axis=1
    ).astype(f32)
    cn = np.full((128, 1), 1.0 / D, f32).astype(bf)
    wq_h = np.ascontiguousarray(np.asarray(Wq, f32)).astype(bf)
    wk_h = np.ascontiguousarray(np.asarray(Wk, f32)).astype(bf)
    wo_h = np.ascontiguousarray(np.asarray(Wo, f32)).astype(bf)

    in_maps = []
    for b in range(B):
        qt = np.ascontiguousarray(np.asarray(Q[b], f32).T).astype(bf)
        idx = np.nonzero(np.asarray(mask)[b] != 0)[0]
        kc = np.zeros((nkp, D), f32)
        kc[: len(idx)] = np.asarray(K[b], f32)[idx]
        indrow = np.zeros((1, nkp), f32)
        indrow[0, : len(idx)] = 1.0
        kt = np.ascontiguousarray(np.vstack([kc.T, indrow])).astype(bf)
        ind = np.ascontiguousarray(indrow.reshape(kt_tiles, 128).T).astype(bf)
        in_maps.append(
            {
                "qt": qt,
                "kt": kt,
                "wq": wq_h,
                "wk": wk_h,
                "wv": wv_h,
                "wo": wo_h,
                "bq": percol(bq),
                "bk": percol(bk),
                "bo": percol(bo),
                "ind": ind,
                "cr": cr,
                "gb": gb,
                "cn": cn,
            }
        )
    return in_maps


def kernel(Q, K, mask, Wq, bq, Wk, bk, Wv, bv, Wo, bo, g0, b0, g1, b1):
    mask = np.asarray(mask)
    kt_tiles = _kt_tiles_for(mask)
    affine = _is_affine(g0, b0, g1, b1)
    nc = _get_nc(kt_tiles, affine)
    in_maps = prep_inputs(
        Q, K, mask, Wq, bq, Wk, bk, Wv, bv, Wo, bo, g0, b0, g1, b1, kt_tiles
    )
    res = run_bass_kernel_spmd(nc, in_maps, list(range(N_CORES)))
    out = np.stack(
        [np.ascontiguousarray(res.results[i]["out"].T) for i in range(N_CORES)]
    )
    return out.astype(np.float32)


# revision 13
# speedup vs baseline: 1.0985x; 1.0985x over previous
"""Trainium2 Bass kernel for the masked-attention block (nn_MAB_61607010894006).

Sharding: data-parallel over batch B=8 across 8 NeuronCores (one batch row
per core, weights replicated, no collectives).

v4 design (vs 162us v2 / 177us v3):
  - ACT (scalar) engine is the fundamental bottleneck: ~5.2M softmax exps
    at 1 elem/cycle/lane @1.2GHz.  Everything else is arranged around
    keeping the ACT exp stream dense and the PE warm (HAM clock gate).
  - Softmax exps merged: scores per (head, ktile) land in a 2-bank
    [128,1024] PSUM tile (2 row-tiled matmuls per chunk), ONE exp each.
  - Software pipeline: window t emits scores(t)+exps(t) interleaved with
    proj(t+1) (early, so the next window's scores aren't gated on the
    proj->cast latency) and A@V(t-1) + normalize(t-1) as PE/DVE filler.
  - Softmax denominators: ride A@V as the 65th v row; po[0:65] is drained
    to SBUF bf16 immediately (frees the PSUM bank); the denom row is
    DMA-hopped (2KB) to partitions 0/32 of a per-pair gather tile so ONE
    Ln + ONE Exp [33,1024] serves a whole head pair (ACT lanes in
    parallel instead of 1-lane [1,512] ops).
  - LN0 sums accumulate in-window (PE filler); LN1/FC epilogue balances
    DVE (relu, applies) vs ACT (squares, rr/rm casts).
"""

import sys

sys.path.insert(0, "/opt/trn_rl_repo")

import numpy as np
import ml_dtypes

import concourse.bass as bass
import concourse.mybir as mybir
import concourse.tile as tile
from concourse.bass_utils import run_bass_kernel_spmd


F32 = mybir.dt.float32
BF16 = mybir.dt.bfloat16
AF = mybir.ActivationFunctionType

B, NQ, NK, D, H, DH = 8, 1024, 1024, 512, 8, 64
EPS = 1e-5
N_CORES = 8


def _split_multi_waits(nc):
    """This toolchain's walrus allows ONE sem wait per TPB instruction; Tile
    can emit several (kernel-tail drain). Hoist extras onto preceding
    single-wait NOPs on the same engine stream (equivalent: in-order issue).
    """
    multi_update = []
    for fn in nc.m.functions:
        for bb in fn.blocks:
            insts = bb.instructions
            new = []
            changed = False
            for inst in insts:
                si = inst.sync_info
                if si is not None and si.on_wait and len(si.on_wait) > 1:
                    waits = list(si.on_wait)
                    for w in waits[:-1]:
                        nop = mybir.InstNoOp(
                            name=f"I-wsplit-{nc.next_id()}", engine=inst.engine
                        )
                        nop.sync_info = mybir.SyncInfo(on_wait=[w], on_update=[])
                        new.append(nop)
                    inst.sync_info = mybir.SyncInfo(
                        on_wait=[waits[-1]], on_update=list(si.on_update)
                    )
                    changed = True
                if si is not None and si.on_update and len(si.on_update) > 1:
                    multi_update.append(inst.name)
                new.append(inst)
            if changed:
                bb.instructions = new
    if multi_update:
        raise RuntimeError(f">1 sem update unsupported: {multi_update[:10]}")


def build_nc(kt_tiles=5, affine=False):
    NKP = kt_tiles * 128  # compacted+padded key/value token count
    nc = bass.Bass()

    qt_d = nc.dram_tensor("qt", [D, NQ], BF16, kind="ExternalInput")
    kt_d = nc.dram_tensor("kt", [D + 1, NKP], BF16, kind="ExternalInput")  # +ind
    wq_d = nc.dram_tensor("wq", [D, D], BF16, kind="ExternalInput")
    wk_d = nc.dram_tensor("wk", [D, D], BF16, kind="ExternalInput")
    wv_d = nc.dram_tensor("wv", [D + 1, D], BF16, kind="ExternalInput")  # +bv row
    wo_d = nc.dram_tensor("wo", [D, D], BF16, kind="ExternalInput")
    bq_d = nc.dram_tensor("bq", [128, 4], F32, kind="ExternalInput")
    bk_d = nc.dram_tensor("bk", [128, 4], F32, kind="ExternalInput")
    bo_d = nc.dram_tensor("bo", [128, 4], F32, kind="ExternalInput")
    ind_d = nc.dram_tensor("ind", [128, kt_tiles], BF16, kind="ExternalInput")
    cr_d = nc.dram_tensor("cr", [33, 128], BF16, kind="ExternalInput")  # ones @0,32
    gb_d = nc.dram_tensor("gb", [128, 16], F32, kind="ExternalInput")  # percol
    cn_d = nc.dram_tensor("cn", [128, 1], BF16, kind="ExternalInput")  # 1/512
    out_d = nc.dram_tensor("out", [D, NQ], BF16, kind="ExternalOutput")

    mult, add = mybir.AluOpType.mult, mybir.AluOpType.add
    amax = mybir.AluOpType.max

    def mm(out, lhsT, rhs, **kw):
        nc.tensor.matmul(out, lhsT, rhs, **kw)

    with tile.TileContext(nc) as tc:
        with (
            tc.tile_pool(name="wp", bufs=1) as wp,
            tc.tile_pool(name="ap", bufs=1) as ap,
            tc.tile_pool(name="sm", bufs=2) as sm,
            tc.tile_pool(name="pp", bufs=1, space="PSUM") as pp,
        ):
            # PSUM: 8 banks.  sx = 2x [128,1024] (4) proj/scores/FC;
            # po = 2x [128,512] (2) A@V + pb + LN broadcasts;
            # spb = 2x [65,512] (2) LN stat sums (live across windows).
            def sx_tile(name):
                return pp.tile([128, 1024], F32, name=name, tag="sx", bufs=2)

            def po_tile(name):
                return pp.tile([128, 512], F32, name=name, tag="po", bufs=2)

            def spb_tile(name):
                return pp.tile([65, 512], F32, name=name, tag="spb", bufs=2)

            # ---- tiles ---------------------------------------------------------
            bq_sb = wp.tile([128, 4], F32, name="bq_sb")
            bk_sb = wp.tile([128, 4], F32, name="bk_sb")
            bo_sb = wp.tile([128, 4], F32, name="bo_sb")
            ind_sb = wp.tile([128, kt_tiles], BF16, name="ind_sb")
            cr_sb = wp.tile([33, 128], BF16, name="cr_sb")
            gb_sb = wp.tile([128, 16], F32, name="gb_sb")
            cn_sb = wp.tile([128, 1], BF16, name="cn_sb")
            eps_sb = wp.tile([1, 1], F32, name="eps_sb")
            nc.vector.memset(eps_sb[:], EPS)
            sum_lhs = cn_sb[:, 0:1]                    # [128,1] bf16 = 1/512
            ones_row = cr_sb[0:1, :]                   # [1,128] bf16 lhsT

            # ---- PE warmup while DMAs stream (HAM un-throttles after ~3.4us
            # of sustained activity; keep it busy until proj(0) can start) ----
            wu_sb = wp.tile([128, 128], BF16, name="wu_sb")
            nc.vector.memset(wu_sb[:], 0.001)
            wu_ps = sx_tile("wu_ps")
            for i in range(140):
                mm(wu_ps[:, 0:128], wu_sb[:], wu_sb[:],
                   start=(i == 0), stop=(i == 139))
            wu_out = wp.tile([1, 1], F32, name="wu_out")
            nc.vector.tensor_copy(wu_out[:], wu_ps[0:1, 0:1])

            # ---- inputs spread over the 3 DMA queues (sync/scalar/gpsimd).
            # Order = first-use order: kt,wk (k-proj), qt,wq (q-proj),
            # wv (v-proj), consts, wo (FC, much later).  Each dma_start costs
            # ~700ns of issue time on its queue, so consts go late. ----------
            wq_sb = wp.tile([128, 4 * D], BF16, name="wq_sb")
            wk_sb = wp.tile([128, 4 * D], BF16, name="wk_sb")
            wv_sb = wp.tile([128, 4 * D], BF16, name="wv_sb")
            wv1_sb = wp.tile([1, D], BF16, name="wv1_sb")
            wo_sb = wp.tile([128, 4 * D], BF16, name="wo_sb")
            kt_sb = wp.tile([128, 4 * NKP], BF16, name="kt_sb")
            kt1_sb = wp.tile([1, NKP], BF16, name="kt1_sb")
            qt_sb = wp.tile([128, 4 * NQ], BF16, name="qt_sb")
            QS = [nc.sync, nc.scalar, nc.gpsimd]
            qi = 0

            def dma(dst, src):
                nonlocal qi
                QS[qi % 3].dma_start(dst, src)
                qi += 1

            for t in range(4):
                dma(kt_sb[:, t * NKP : (t + 1) * NKP],
                    kt_d[t * 128 : (t + 1) * 128, :])
            for t in range(4):
                dma(wk_sb[:, t * D : (t + 1) * D],
                    wk_d[t * 128 : (t + 1) * 128, :])
            for t in range(4):
                dma(qt_sb[:, t * NQ : (t + 1) * NQ],
                    qt_d[t * 128 : (t + 1) * 128, :])
            for t in range(4):
                dma(wq_sb[:, t * D : (t + 1) * D],
                    wq_d[t * 128 : (t + 1) * 128, :])
            for t in range(4):
                dma(wv_sb[:, t * D : (t + 1) * D],
                    wv_d[t * 128 : (t + 1) * 128, :])
            dma(wv1_sb[:, :], wv_d[D : D + 1, :])
            dma(kt1_sb[:, :], kt_d[D : D + 1, :])
            dma(bq_sb[:], bq_d[:])
            dma(bk_sb[:], bk_d[:])
            dma(bo_sb[:], bo_d[:])
            dma(ind_sb[:], ind_d[:])
            dma(cr_sb[:], cr_d[:])
            dma(gb_sb[:], gb_d[:])
            dma(cn_sb[:], cn_d[:])
            for t in range(4):
                dma(wo_sb[:, t * D : (t + 1) * D],
                    wo_d[t * 128 : (t + 1) * 128, :])

            # ---- persistent activations --------------------------------------
            q_bf = ap.tile([128, 4 * NQ], BF16, name="q_bf")
            k_bf = ap.tile([128, 4 * NKP], BF16, name="k_bf")
            v_sb = ap.tile([128, kt_tiles * (8 * 65)], BF16, name="v_sb")
            v_ones = v_sb.rearrange("p (v h x) -> p v h x", v=kt_tiles, h=8)[
                :, :, :, 64
            ]
            nc.vector.tensor_copy(
                v_ones,
                ind_sb.rearrange("p (v a) -> p v a", a=1)
                .broadcast_to([128, kt_tiles, 8]),
            )

            # ---- projection emitters (k and q separately, for interleave) ----
            def proj_k(t):
                ps_k = sx_tile(f"ps_k{t}")
                kchunks = [(0, min(NKP, 512))] + (
                    [(512, NKP - 512)] if NKP > 512 else []
                )
                for kc in range(4):
                    for cs, cw in kchunks:
                        mm(
                            ps_k[:, cs : cs + cw],
                            wk_sb[:, kc * D + t * 128 : kc * D + (t + 1) * 128],
                            kt_sb[:, kc * NKP + cs : kc * NKP + cs + cw],
                            start=(kc == 0),
                            stop=(kc == 3),
                        )
                nc.vector.tensor_scalar_add(
                    k_bf[:, t * NKP : (t + 1) * NKP],
                    ps_k[:, 0:NKP],
                    bk_sb[:, t : t + 1],
                )

            def proj_q(t):
                ps_q = sx_tile(f"ps_q{t}")
                for kc in range(4):
                    for c in range(2):
                        mm(
                            ps_q[:, c * 512 : c * 512 + 512],
                            wq_sb[:, kc * D + t * 128 : kc * D + (t + 1) * 128],
                            qt_sb[:, kc * NQ + c * 512 : kc * NQ + c * 512 + 512],
                            start=(kc == 0),
                            stop=(kc == 3),
                        )
                nc.vector.tensor_scalar_add(
                    q_bf[:, t * NQ : (t + 1) * NQ],
                    ps_q[:],
                    bq_sb[:, t : t + 1],
                )

            def vproj(vt):
                ps_v = sx_tile(f"ps_v{vt}")
                for kc in range(4):
                    mm(
                        ps_v[:, 0:512],
                        kt_sb[:, kc * NKP + vt * 128 : kc * NKP + (vt + 1) * 128],
                        wv_sb[:, kc * D : (kc + 1) * D],
                        start=(kc == 0),
                        stop=False,
                    )
                mm(
                    ps_v[:, 0:512],
                    kt1_sb[0:1, vt * 128 : (vt + 1) * 128],
                    wv1_sb[0:1, :],
                    start=False,
                    stop=True,
                )
                v_dst = v_sb[:, vt * 520 : (vt + 1) * 520].rearrange(
                    "p (h x) -> p h x", h=8
                )[:, :, 0:64]
                nc.scalar.copy(v_dst, ps_v[:, 0:512].rearrange("p (h x) -> p h x", h=8))

            o_bf = ap.tile([128, 4 * NQ], BF16, name="o_bf")
            at_tiles = {}  # t -> {sub: [tiles]}
            av_tiles = {}  # h -> av_sb [65, NQ]
            dg_tiles = {}  # pair -> [33, NQ]
            sqt_tiles = []
            sts0 = [spb_tile(f"stl0{c}") for c in range(2)]

            def scores_i(t, i):
                """Scores + exp for ktile i of head pair (2t, 2t+1)."""
                pss = {}
                for sub in range(2):
                    pss[sub] = sx_tile(f"s{t}_{i}_{sub}")
                for c in range(2):
                    for sub in range(2):
                        rh = sub * 64
                        mm(
                            pss[sub][:, c * 512 : c * 512 + 512],
                            k_bf[rh : rh + 64,
                                 t * NKP + i * 128 : t * NKP + (i + 1) * 128],
                            q_bf[rh : rh + 64,
                                 t * NQ + c * 512 : t * NQ + c * 512 + 512],
                            start=True,
                            stop=True,
                        )
                for sub in range(2):
                    nc.scalar.activation(
                        at_tiles[t][sub][i][:], pss[sub][:], AF.Exp, scale=0.125
                    )

            def av_chunk(t, sub, c):
                """A@V for (head 2t+sub, q-chunk c): 5 mms + drain to SBUF."""
                h = 2 * t + sub
                if sub == 0 and c == 0:
                    dg_tiles[t] = sm.tile([33, NQ], BF16, name=f"dg{t}",
                                          tag="dg", bufs=2)
                if c == 0:
                    av_tiles[h] = sm.tile([65, NQ], BF16, name=f"av{h}",
                                          tag="avs", bufs=4)
                po = po_tile(f"po{h}_{c}")
                for i in range(kt_tiles):
                    mm(
                        po[0:65, :],
                        v_sb[:, i * 520 + h * 65 : i * 520 + (h + 1) * 65],
                        at_tiles[t][sub][i][:, c * 512 : (c + 1) * 512],
                        start=(i == 0),
                        stop=(i == kt_tiles - 1),
                    )
                nc.vector.tensor_copy(
                    av_tiles[h][:, c * 512 : c * 512 + 512], po[0:65, :]
                )
                if c == 1:
                    # hop the denom row (2KB) to partition 32*sub of pair tile
                    nc.sync.dma_start(
                        dg_tiles[t][32 * sub : 32 * sub + 1, :],
                        av_tiles[h][64:65, :],
                    )

            def norm_lnexp(t):
                """1/denom for pair t: ONE Ln + ONE Exp over [33,1024]."""
                dg = dg_tiles[t]
                lnd = sm.tile([33, NQ], F32, name=f"lnd{t}", tag="lnd", bufs=2)
                nc.scalar.activation(lnd[:], dg[:], AF.Ln)
                rinv = sm.tile([33, NQ], BF16, name=f"ri{t}", tag="ri", bufs=2)
                nc.scalar.activation(rinv[:], lnd[:], AF.Exp, scale=-1.0)
                dg_tiles[t] = rinv

            def norm_apply(t):
                """Broadcast + normalize + residual add, sqt, LN0 sums."""
                tsl = slice(t * NQ, (t + 1) * NQ)
                rinv = dg_tiles[t]
                for sub in range(2):
                    h = 2 * t + sub
                    rp = 32 * sub
                    avn = sm.tile([64, NQ], BF16, name=f"avn{h}", tag="avn",
                                  bufs=2)
                    for c in range(2):
                        csl = slice(c * 512, c * 512 + 512)
                        pb = po_tile(f"pb{h}_{c}")
                        mm(pb[0:64, :], cr_sb[rp : rp + 1, 0:64],
                           rinv[rp : rp + 1, csl], start=True, stop=True)
                        nc.vector.tensor_mul(
                            avn[:, csl], av_tiles[h][0:64, csl], pb[0:64, :]
                        )
                    if sub == 0:
                        nc.vector.tensor_add(
                            o_bf[0:64, tsl], avn[:], q_bf[0:64, tsl]
                        )
                    else:
                        av2 = sm.tile([128, NQ], BF16, name=f"av2_{h}",
                                      tag="av2", bufs=2)
                        nc.gpsimd.dma_start(av2[64:128, :], avn[:])
                        nc.vector.tensor_add(
                            o_bf[64:128, tsl], av2[64:128, :], q_bf[64:128, tsl]
                        )
                sqt = sm.tile([128, NQ], BF16, name=f"sqt{t}", tag="sqt", bufs=4)
                sqt_tiles.append(sqt)
                nc.vector.tensor_mul(sqt[:], o_bf[:, tsl], o_bf[:, tsl])
                # LN0 partial sums for block t (PE filler, col-tiled rows 0/64)
                for c in range(2):
                    csl = slice(t * NQ + c * 512, t * NQ + c * 512 + 512)
                    mm(sts0[c][0:1, :], sum_lhs, o_bf[:, csl],
                       start=(t == 0), stop=(t == 3))
                    mm(sts0[c][64:65, :], sum_lhs,
                       sqt[:, c * 512 : c * 512 + 512],
                       start=(t == 0), stop=(t == 3))

            # ---- pipelined main loop -----------------------------------------
            # window t: scores(t)/exps(t) paced by ACT, with A@V(t-1) in
            # 5-mm chunks + proj(t+1) as PE filler between score tiles.
            proj_k(0)
            proj_q(0)
            for t in range(4):
                at_tiles[t] = {
                    sub: [
                        ap.tile([128, NQ], BF16, name=f"at{t}_{i}_{sub}",
                                tag="at", bufs=20)
                        for i in range(kt_tiles)
                    ]
                    for sub in range(2)
                }
                # filler schedule: one item after each scores tile
                filler = []
                if t == 0:
                    filler = [lambda v=vt: vproj(v) for vt in range(kt_tiles)]
                    filler += [lambda: proj_k(1), lambda: proj_q(1)]
                else:
                    filler = [
                        lambda: av_chunk(t - 1, 0, 0),
                        lambda: av_chunk(t - 1, 0, 1),
                        lambda: av_chunk(t - 1, 1, 0),
                        lambda: av_chunk(t - 1, 1, 1),
                        lambda: norm_lnexp(t - 1),
                        lambda: norm_apply(t - 1),
                    ]
                    if t < 3:
                        filler.insert(2, lambda: proj_k(t + 1))
                        filler.insert(5, lambda: proj_q(t + 1))
                fi = 0
                for i in range(kt_tiles):
                    scores_i(t, i)
                    # ~1-2 filler items between score tiles
                    take = 2 if i >= kt_tiles - 2 else 1
                    for _ in range(take):
                        if fi < len(filler):
                            filler[fi]()
                            fi += 1
                while fi < len(filler):
                    filler[fi]()
                    fi += 1
            for sub in range(2):
                for c in range(2):
                    av_chunk(3, sub, c)
            norm_lnexp(3)
            norm_apply(3)

            # ---- LN stats + broadcast, one c-chunk at a time ------------------
            def ln_stats_c(sts, c, tag):
                """Returns (rr_c, rm_c): [128,512] bf16 broadcasts for chunk c."""
                csl = slice(c * 512, (c + 1) * 512)
                var = sm.tile([1, 512], F32, name=f"var{tag}{c}", tag="var",
                              bufs=2)
                mu = sm.tile([1, 512], F32, name=f"mu{tag}{c}", tag="mu", bufs=2)
                nc.scalar.activation(var[:], sts[c][0:1, :], AF.Square)
                nc.vector.tensor_sub(var[:], sts[c][64:65, :], var[:])
                nc.vector.tensor_copy(mu[:], sts[c][0:1, :])
                rstd = sm.tile([1, 512], BF16, name=f"rs{tag}{c}", tag="rs",
                               bufs=2)
                murm = sm.tile([1, 512], BF16, name=f"mm{tag}{c}", tag="mm2",
                               bufs=2)
                nc.scalar.activation(var[:], var[:], AF.Ln,
                                     bias=eps_sb[0:1, 0:1])
                nc.scalar.activation(rstd[:], var[:], AF.Exp, scale=-0.5)
                nc.vector.tensor_mul(murm[:], mu[:], rstd[:])
                rr = sm.tile([128, 512], BF16, name=f"rrb{tag}{c}", tag="rrb",
                             bufs=2)
                rm = sm.tile([128, 512], BF16, name=f"rmb{tag}{c}", tag="rmb",
                             bufs=2)
                rr_ps = po_tile(f"rr{tag}{c}")
                mm(rr_ps[:, :], ones_row, rstd[:], start=True, stop=True)
                nc.scalar.copy(rr[:], rr_ps[:, :])
                rm_ps = po_tile(f"rm{tag}{c}")
                mm(rm_ps[:, :], ones_row, murm[:], start=True, stop=True)
                nc.scalar.copy(rm[:], rm_ps[:, :])
                return rr, rm

            # ---- LN0 apply + FC + LN1, pipelined per c-chunk ------------------
            ot0 = ap.tile([128, 4 * NQ], BF16, name="ot0")
            o1 = ap.tile([128, 4 * NQ], BF16, name="o1")
            sq1_tiles = [
                sm.tile([128, NQ], BF16, name=f"sq1t{ot}", tag="sqt", bufs=4)
                for ot in range(4)
            ]
            sts1 = [spb_tile(f"stl1{c}") for c in range(2)]
            for c in range(2):
                rr0, rm0 = ln_stats_c(sts0, c, "l0")
                for t in range(4):
                    sl = slice(t * NQ + c * 512, t * NQ + c * 512 + 512)
                    nc.vector.tensor_mul(ot0[:, sl], o_bf[:, sl], rr0[:])
                    nc.vector.tensor_sub(ot0[:, sl], ot0[:, sl], rm0[:])
                    if affine:
                        nc.vector.tensor_scalar(
                            ot0[:, sl], ot0[:, sl],
                            gb_sb[:, 0 + t : 0 + t + 1],
                            gb_sb[:, 4 + t : 4 + t + 1],
                            mult, add,
                        )
                # FC for chunk c (needs ot0[:, *, c] for all 4 t-blocks)
                for ot in range(4):
                    ps_f = sx_tile(f"psf{ot}_{c}")
                    for ft in range(4):
                        mm(
                            ps_f[:, 0:512],
                            wo_sb[:, ft * D + ot * 128 : ft * D + (ot + 1) * 128],
                            ot0[:, ft * NQ + c * 512 : ft * NQ + c * 512 + 512],
                            start=(ft == 0),
                            stop=(ft == 3),
                        )
                    csl = slice(ot * NQ + c * 512, ot * NQ + c * 512 + 512)
                    rl = sm.tile([128, 512], BF16, name=f"rl{ot}{c}", tag="rl",
                                 bufs=2)
                    nc.vector.tensor_scalar(
                        rl[:], ps_f[:, 0:512], bo_sb[:, ot : ot + 1], 0.0,
                        add, amax,
                    )
                    nc.vector.tensor_add(o1[:, csl], ot0[:, csl], rl[:])
                    sq1t = sq1_tiles[ot]
                    nc.scalar.activation(
                        sq1t[:, c * 512 : c * 512 + 512], o1[:, csl], AF.Square
                    )
                    mm(sts1[c][0:1, :], sum_lhs, o1[:, csl],
                       start=(ot == 0), stop=(ot == 3))
                    mm(sts1[c][64:65, :], sum_lhs,
                       sq1t[:, c * 512 : c * 512 + 512],
                       start=(ot == 0), stop=(ot == 3))

            # ---- LN1 -> out ---------------------------------------------------
            otout = ap.tile([128, 4 * NQ], BF16, name="otout")
            oq = 0
            for c in range(2):
                rr1, rm1 = ln_stats_c(sts1, c, "l1")
                for t in range(4):
                    sl = slice(t * NQ + c * 512, t * NQ + c * 512 + 512)
                    tmp = sm.tile([128, 512], BF16, name=f"tmp{t}{c}", tag="rl",
                                  bufs=2)
                    nc.vector.tensor_mul(tmp[:], o1[:, sl], rr1[:])
                    if affine:
                        nc.vector.tensor_sub(tmp[:], tmp[:], rm1[:])
                        nc.vector.tensor_scalar(
                            otout[:, sl], tmp[:],
                            gb_sb[:, 8 + t : 8 + t + 1],
                            gb_sb[:, 12 + t : 12 + t + 1],
                            mult, add,
                        )
                    else:
                        nc.vector.tensor_sub(otout[:, sl], tmp[:], rm1[:])
                    QS[oq % 3].dma_start(
                        out_d[t * 128 : (t + 1) * 128, c * 512 : c * 512 + 512],
                        otout[:, sl],
                    )
                    oq += 1

    _split_multi_waits(nc)
    return nc


_nc_cache = {}


def _get_nc(kt_tiles=5, affine=False):
    key = (kt_tiles, affine)
    if key not in _nc_cache:
        _nc_cache[key] = build_nc(kt_tiles, affine)
    return _nc_cache[key]


def _kt_tiles_for(mask):
    n = int(max(int((mask[b] != 0).sum()) for b in range(mask.shape[0])))
    return max(1, (n + 127) // 128)


def _is_affine(g0, b0, g1, b1):
    return not (
        np.all(np.asarray(g0) == 1.0)
        and np.all(np.asarray(b0) == 0.0)
        and np.all(np.asarray(g1) == 1.0)
        and np.all(np.asarray(b1) == 0.0)
    )


def prep_inputs(Q, K, mask, Wq, bq, Wk, bk, Wv, bv, Wo, bo, g0, b0, g1, b1,
                kt_tiles=None):
    f32, bf = np.float32, ml_dtypes.bfloat16
    if kt_tiles is None:
        kt_tiles = _kt_tiles_for(np.asarray(mask))
    nkp = kt_tiles * 128

    def percol(v):  # [512] feature vector -> [128, 4] per-partition layout
        return np.ascontiguousarray(np.asarray(v, f32).reshape(4, 128).T)

    wv_h = np.ascontiguousarray(
        np.vstack([np.asarray(Wv, f32), np.asarray(bv, f32)[None, :]])
    ).astype(bf)
    cr = np.zeros((33, 128), f32)
    cr[0, :] = 1.0
    cr[32, :] = 1.0
    cr = cr.astype(bf)
    gb = np.concatenate(
        [percol(g0), percol(b0), percol(g1), percol(b1)], axis=1
    ).astype(f32)
    cn = np.full((128, 1), 1.0 / D, f32).astype(bf)
    wq_h = np.ascontiguousarray(np.asarray(Wq, f32)).astype(bf)
    wk_h = np.ascontiguousarray(np.asarray(Wk, f32)).astype(bf)
    wo_h = np.ascontiguousarray(np.asarray(Wo, f32)).astype(bf)

    in_maps = []
    for b in range(B):
        qt = np.ascontiguousarray(np.asarray(Q[b], f32).T).astype(bf)
        idx = np.nonzero(np.asarray(mask)[b] != 0)[0]
        kc = np.zeros((nkp, D), f32)
        kc[: len(idx)] = np.asarray(K[b], f32)[idx]
        indrow = np.zeros((1, nkp), f32)
        indrow[0, : len(idx)] = 1.0
        kt = np.ascontiguousarray(np.vstack([kc.T, indrow])).astype(bf)
        ind = np.ascontiguousarray(indrow.reshape(kt_tiles, 128).T).astype(bf)
        in_maps.append(
            {
                "qt": qt,
                "kt": kt,
                "wq": wq_h,
                "wk": wk_h,
                "wv": wv_h,
                "wo": wo_h,
                "bq": percol(bq),
                "bk": percol(bk),
                "bo": percol(bo),
                "ind": ind,
                "cr": cr,
                "gb": gb,
                "cn": cn,
            }
        )
    return in_maps


def kernel(Q, K, mask, Wq, bq, Wk, bk, Wv, bv, Wo, bo, g0, b0, g1, b1):
    mask = np.asarray(mask)
    kt_tiles = _kt_tiles_for(mask)
    affine = _is_affine(g0, b0, g1, b1)
    nc = _get_nc(kt_tiles, affine)
    in_maps = prep_inputs(
        Q, K, mask, Wq, bq, Wk, bk, Wv, bv, Wo, bo, g0, b0, g1, b1, kt_tiles
    )
    res = run_bass_kernel_spmd(nc, in_maps, list(range(N_CORES)))
    out = np.stack(
        [np.ascontiguousarray(res.results[i]["out"].T) for i in range(N_CORES)]
    )
    return out.astype(np.float32)


# revision 19
# speedup vs baseline: 1.2798x; 1.1651x over previous
"""Trainium2 Bass kernel for the masked-attention block (nn_MAB_61607010894006).

Sharding: data-parallel over batch B=8 across 8 NeuronCores (one batch row
per core, weights replicated, no collectives).

v4 design (vs 162us v2 / 177us v3):
  - ACT (scalar) engine is the fundamental bottleneck: ~5.2M softmax exps
    at 1 elem/cycle/lane @1.2GHz.  Everything else is arranged around
    keeping the ACT exp stream dense and the PE warm (HAM clock gate).
  - Softmax exps merged: scores per (head, ktile) land in a 2-bank
    [128,1024] PSUM tile (2 row-tiled matmuls per chunk), ONE exp each.
  - Software pipeline: window t emits scores(t)+exps(t) interleaved with
    proj(t+1) (early, so the next window's scores aren't gated on the
    proj->cast latency) and A@V(t-1) + normalize(t-1) as PE/DVE filler.
  - Softmax denominators: ride A@V as the 65th v row; po[0:65] is drained
    to SBUF bf16 immediately (frees the PSUM bank); the denom row is
    DMA-hopped (2KB) to partitions 0/32 of a per-pair gather tile so ONE
    Ln + ONE Exp [33,1024] serves a whole head pair (ACT lanes in
    parallel instead of 1-lane [1,512] ops).
  - LN0 sums accumulate in-window (PE filler); LN1/FC epilogue balances
    DVE (relu, applies) vs ACT (squares, rr/rm casts).
"""

import sys

sys.path.insert(0, "/opt/trn_rl_repo")

import numpy as np
import ml_dtypes

import concourse.bass as bass
import concourse.mybir as mybir
import concourse.tile as tile
from concourse.bass_utils import run_bass_kernel_spmd


F32 = mybir.dt.float32
BF16 = mybir.dt.bfloat16
AF = mybir.ActivationFunctionType

B, NQ, NK, D, H, DH = 8, 1024, 1024, 512, 8, 64
EPS = 1e-5
N_CORES = 8


def _split_multi_waits(nc):
    """This toolchain's walrus allows ONE sem wait per TPB instruction; Tile
    can emit several (kernel-tail drain). Hoist extras onto preceding
    single-wait NOPs on the same engine stream (equivalent: in-order issue).
    """
    multi_update = []
    for fn in nc.m.functions:
        for bb in fn.blocks:
            insts = bb.instructions
            new = []
            changed = False
            for inst in insts:
                si = inst.sync_info
                if si is not None and si.on_wait and len(si.on_wait) > 1:
                    waits = list(si.on_wait)
                    for w in waits[:-1]:
                        nop = mybir.InstNoOp(
                            name=f"I-wsplit-{nc.next_id()}", engine=inst.engine
                        )
                        nop.sync_info = mybir.SyncInfo(on_wait=[w], on_update=[])
                        new.append(nop)
                    inst.sync_info = mybir.SyncInfo(
                        on_wait=[waits[-1]], on_update=list(si.on_update)
                    )
                    changed = True
                if si is not None and si.on_update and len(si.on_update) > 1:
                    multi_update.append(inst.name)
                new.append(inst)
            if changed:
                bb.instructions = new
    if multi_update:
        raise RuntimeError(f">1 sem update unsupported: {multi_update[:10]}")


def build_nc(kt_tiles=5, affine=False):
    NKP = kt_tiles * 128  # compacted+padded key/value token count
    nc = bass.Bass()

    qt_d = nc.dram_tensor("qt", [D, NQ], BF16, kind="ExternalInput")
    kt_d = nc.dram_tensor("kt", [D + 1, NKP], BF16, kind="ExternalInput")  # +ind
    wq_d = nc.dram_tensor("wq", [D, D], BF16, kind="ExternalInput")
    wk_d = nc.dram_tensor("wk", [D, D], BF16, kind="ExternalInput")
    wv_d = nc.dram_tensor("wv", [D + 1, D], BF16, kind="ExternalInput")  # +bv row
    wo_d = nc.dram_tensor("wo", [D, D], BF16, kind="ExternalInput")
    bq_d = nc.dram_tensor("bq", [128, 4], F32, kind="ExternalInput")
    bk_d = nc.dram_tensor("bk", [128, 4], F32, kind="ExternalInput")
    bo_d = nc.dram_tensor("bo", [128, 4], F32, kind="ExternalInput")
    ind_d = nc.dram_tensor("ind", [128, kt_tiles], BF16, kind="ExternalInput")
    cr_d = nc.dram_tensor("cr", [33, 128], BF16, kind="ExternalInput")  # ones @0,32
    gb_d = nc.dram_tensor("gb", [128, 16], F32, kind="ExternalInput")  # percol
    cn_d = nc.dram_tensor("cn", [128, 1], BF16, kind="ExternalInput")  # 1/512
    out_d = nc.dram_tensor("out", [D, NQ], BF16, kind="ExternalOutput")

    mult, add = mybir.AluOpType.mult, mybir.AluOpType.add
    amax = mybir.AluOpType.max

    def mm(out, lhsT, rhs, **kw):
        nc.tensor.matmul(out, lhsT, rhs, **kw)

    with tile.TileContext(nc) as tc:
        with (
            tc.tile_pool(name="wp", bufs=1) as wp,
            tc.tile_pool(name="ap", bufs=1) as ap,
            tc.tile_pool(name="sm", bufs=2) as sm,
            tc.tile_pool(name="pp", bufs=1, space="PSUM") as pp,
        ):
            # PSUM: 8 banks.  sx = 2x [128,1024] (4) proj/scores/FC;
            # po = 2x [128,512] (2) A@V + pb + LN broadcasts;
            # spb = 2x [65,512] (2) LN stat sums (live across windows).
            def sx_tile(name):
                return pp.tile([128, 1024], F32, name=name, tag="sx", bufs=2)

            def po_tile(name):
                return pp.tile([128, 512], F32, name=name, tag="po", bufs=2)

            def spb_tile(name):
                return pp.tile([65, 512], F32, name=name, tag="spb", bufs=2)

            # ---- tiles ---------------------------------------------------------
            bq_sb = wp.tile([128, 4], F32, name="bq_sb")
            bk_sb = wp.tile([128, 4], F32, name="bk_sb")
            bo_sb = wp.tile([128, 4], F32, name="bo_sb")
            ind_sb = wp.tile([128, kt_tiles], BF16, name="ind_sb")
            cr_sb = wp.tile([33, 128], BF16, name="cr_sb")
            gb_sb = wp.tile([128, 16], F32, name="gb_sb")
            cn_sb = wp.tile([128, 1], BF16, name="cn_sb")
            eps_sb = wp.tile([1, 1], F32, name="eps_sb")
            nc.vector.memset(eps_sb[:], EPS)
            sum_lhs = cn_sb[:, 0:1]                    # [128,1] bf16 = 1/512
            ones_row = cr_sb[0:1, :]                   # [1,128] bf16 lhsT

            # ---- PE warmup while DMAs stream (HAM un-throttles after ~3.4us
            # of sustained activity; keep it busy until proj(0) can start) ----
            wu_sb = wp.tile([128, 128], BF16, name="wu_sb")
            nc.vector.memset(wu_sb[:], 0.001)
            wu_ps = sx_tile("wu_ps")
            for i in range(40):
                mm(wu_ps[:, 0:128], wu_sb[:], wu_sb[:],
                   start=(i == 0), stop=(i == 39))
            wu_out = wp.tile([1, 1], F32, name="wu_out")
            nc.vector.tensor_copy(wu_out[:], wu_ps[0:1, 0:1])

            # ---- inputs spread over the 3 DMA queues (sync/scalar/gpsimd).
            # Order = first-use order: kt,wk (k-proj), qt,wq (q-proj),
            # wv (v-proj), consts, wo (FC, much later).  Each dma_start costs
            # ~700ns of issue time on its queue, so consts go late. ----------
            wq_sb = wp.tile([128, 4 * D], BF16, name="wq_sb")
            wk_sb = wp.tile([128, 4 * D], BF16, name="wk_sb")
            wv_sb = wp.tile([128, 4 * D], BF16, name="wv_sb")
            wv1_sb = wp.tile([1, D], BF16, name="wv1_sb")
            wo_sb = wp.tile([128, 4 * D], BF16, name="wo_sb")
            kt_sb = wp.tile([128, 4 * NKP], BF16, name="kt_sb")
            kt1_sb = wp.tile([1, NKP], BF16, name="kt1_sb")
            qt_sb = wp.tile([128, 4 * NQ], BF16, name="qt_sb")
            QS = [nc.sync, nc.scalar, nc.gpsimd]
            qi = 0

            def dma(dst, src):
                nonlocal qi
                QS[qi % 3].dma_start(dst, src)
                qi += 1

            for t in range(4):
                dma(kt_sb[:, t * NKP : (t + 1) * NKP],
                    kt_d[t * 128 : (t + 1) * 128, :])
            for t in range(4):
                dma(wk_sb[:, t * D : (t + 1) * D],
                    wk_d[t * 128 : (t + 1) * 128, :])
            for t in range(4):
                dma(wv_sb[:, t * D : (t + 1) * D],
                    wv_d[t * 128 : (t + 1) * 128, :])
            dma(wv1_sb[:, :], wv_d[D : D + 1, :])
            dma(kt1_sb[:, :], kt_d[D : D + 1, :])
            for t in range(4):
                dma(qt_sb[:, t * NQ : (t + 1) * NQ],
                    qt_d[t * 128 : (t + 1) * 128, :])
            for t in range(4):
                dma(wq_sb[:, t * D : (t + 1) * D],
                    wq_d[t * 128 : (t + 1) * 128, :])
            dma(bq_sb[:], bq_d[:])
            dma(bk_sb[:], bk_d[:])
            dma(bo_sb[:], bo_d[:])
            dma(ind_sb[:], ind_d[:])
            dma(cr_sb[:], cr_d[:])
            dma(gb_sb[:], gb_d[:])
            dma(cn_sb[:], cn_d[:])
            for t in range(4):
                dma(wo_sb[:, t * D : (t + 1) * D],
                    wo_d[t * 128 : (t + 1) * 128, :])

            # ---- persistent activations --------------------------------------
            q_bf = ap.tile([128, 4 * NQ], BF16, name="q_bf")
            k_bf = ap.tile([128, 4 * NKP], BF16, name="k_bf")
            v_sb = ap.tile([128, kt_tiles * (8 * 65)], BF16, name="v_sb")
            v_ones = v_sb.rearrange("p (v h x) -> p v h x", v=kt_tiles, h=8)[
                :, :, :, 64
            ]
            nc.vector.tensor_copy(
                v_ones,
                ind_sb.rearrange("p (v a) -> p v a", a=1)
                .broadcast_to([128, kt_tiles, 8]),
            )

            # ---- projection emitters (k and q separately, for interleave) ----
            def proj_k(t):
                ps_k = sx_tile(f"ps_k{t}")
                kchunks = [(0, min(NKP, 512))] + (
                    [(512, NKP - 512)] if NKP > 512 else []
                )
                for kc in range(4):
                    for cs, cw in kchunks:
                        mm(
                            ps_k[:, cs : cs + cw],
                            wk_sb[:, kc * D + t * 128 : kc * D + (t + 1) * 128],
                            kt_sb[:, kc * NKP + cs : kc * NKP + cs + cw],
                            start=(kc == 0),
                            stop=(kc == 3),
                        )
                nc.vector.tensor_scalar_add(
                    k_bf[:, t * NKP : (t + 1) * NKP],
                    ps_k[:, 0:NKP],
                    bk_sb[:, t : t + 1],
                )

            def proj_q(t):
                ps_q = sx_tile(f"ps_q{t}")
                for kc in range(4):
                    for c in range(2):
                        mm(
                            ps_q[:, c * 512 : c * 512 + 512],
                            wq_sb[:, kc * D + t * 128 : kc * D + (t + 1) * 128],
                            qt_sb[:, kc * NQ + c * 512 : kc * NQ + c * 512 + 512],
                            start=(kc == 0),
                            stop=(kc == 3),
                        )
                nc.vector.tensor_scalar_add(
                    q_bf[:, t * NQ : (t + 1) * NQ],
                    ps_q[:],
                    bq_sb[:, t : t + 1],
                )

            def vproj(vt):
                ps_v = sx_tile(f"ps_v{vt}")
                for kc in range(4):
                    mm(
                        ps_v[:, 0:512],
                        kt_sb[:, kc * NKP + vt * 128 : kc * NKP + (vt + 1) * 128],
                        wv_sb[:, kc * D : (kc + 1) * D],
                        start=(kc == 0),
                        stop=False,
                    )
                mm(
                    ps_v[:, 0:512],
                    kt1_sb[0:1, vt * 128 : (vt + 1) * 128],
                    wv1_sb[0:1, :],
                    start=False,
                    stop=True,
                )
                v_dst = v_sb[:, vt * 520 : (vt + 1) * 520].rearrange(
                    "p (h x) -> p h x", h=8
                )[:, :, 0:64]
                nc.scalar.copy(v_dst, ps_v[:, 0:512].rearrange("p (h x) -> p h x", h=8))

            o_bf = ap.tile([128, 4 * NQ], BF16, name="o_bf")
            at_tiles = {}  # t -> {sub: [tiles]}
            av_tiles = {}  # h -> av_sb [65, NQ]
            dg_tiles = {}  # pair -> [33, NQ]
            sqt_tiles = []
            sts0 = [spb_tile(f"stl0{c}") for c in range(2)]

            def scores_i(t, i):
                """Scores + exp for ktile i of head pair (2t, 2t+1)."""
                pss = {}
                for sub in range(2):
                    pss[sub] = sx_tile(f"s{t}_{i}_{sub}")
                for c in range(2):
                    for sub in range(2):
                        rh = sub * 64
                        mm(
                            pss[sub][:, c * 512 : c * 512 + 512],
                            k_bf[rh : rh + 64,
                                 t * NKP + i * 128 : t * NKP + (i + 1) * 128],
                            q_bf[rh : rh + 64,
                                 t * NQ + c * 512 : t * NQ + c * 512 + 512],
                            start=True,
                            stop=True,
                        )
                for sub in range(2):
                    nc.scalar.activation(
                        at_tiles[t][sub][i][:], pss[sub][:], AF.Exp, scale=0.125
                    )

            def av_chunk(t, sub, c):
                """A@V for (head 2t+sub, q-chunk c): 5 mms + drain to SBUF."""
                h = 2 * t + sub
                if sub == 0 and c == 0:
                    dg_tiles[t] = sm.tile([33, NQ], BF16, name=f"dg{t}",
                                          tag="dg", bufs=2)
                if c == 0:
                    av_tiles[h] = sm.tile([65, NQ], BF16, name=f"av{h}",
                                          tag="avs", bufs=4)
                po = po_tile(f"po{h}_{c}")
                for i in range(kt_tiles):
                    mm(
                        po[0:65, :],
                        v_sb[:, i * 520 + h * 65 : i * 520 + (h + 1) * 65],
                        at_tiles[t][sub][i][:, c * 512 : (c + 1) * 512],
                        start=(i == 0),
                        stop=(i == kt_tiles - 1),
                    )
                nc.vector.tensor_copy(
                    av_tiles[h][:, c * 512 : c * 512 + 512], po[0:65, :]
                )
                if c == 1:
                    # hop the denom row (2KB) to partition 32*sub of pair tile
                    nc.sync.dma_start(
                        dg_tiles[t][32 * sub : 32 * sub + 1, :],
                        av_tiles[h][64:65, :],
                    )

            def norm_lnexp(t):
                """1/denom for pair t: ONE Ln + ONE Exp over [33,1024]."""
                dg = dg_tiles[t]
                lnd = sm.tile([33, NQ], F32, name=f"lnd{t}", tag="lnd", bufs=2)
                nc.scalar.activation(lnd[:], dg[:], AF.Ln)
                rinv = sm.tile([33, NQ], BF16, name=f"ri{t}", tag="ri", bufs=2)
                nc.scalar.activation(rinv[:], lnd[:], AF.Exp, scale=-1.0)
                dg_tiles[t] = rinv

            def norm_apply(t):
                """Broadcast + normalize + residual add, sqt, LN0 sums."""
                tsl = slice(t * NQ, (t + 1) * NQ)
                rinv = dg_tiles[t]
                for sub in range(2):
                    h = 2 * t + sub
                    rp = 32 * sub
                    avn = sm.tile([64, NQ], BF16, name=f"avn{h}", tag="avn",
                                  bufs=2)
                    for c in range(2):
                        csl = slice(c * 512, c * 512 + 512)
                        pb = po_tile(f"pb{h}_{c}")
                        mm(pb[0:64, :], cr_sb[rp : rp + 1, 0:64],
                           rinv[rp : rp + 1, csl], start=True, stop=True)
                        nc.vector.tensor_mul(
                            avn[:, csl], av_tiles[h][0:64, csl], pb[0:64, :]
                        )
                    if sub == 0:
                        nc.vector.tensor_add(
                            o_bf[0:64, tsl], avn[:], q_bf[0:64, tsl]
                        )
                    else:
                        av2 = sm.tile([128, NQ], BF16, name=f"av2_{h}",
                                      tag="av2", bufs=2)
                        nc.gpsimd.dma_start(av2[64:128, :], avn[:])
                        nc.vector.tensor_add(
                            o_bf[64:128, tsl], av2[64:128, :], q_bf[64:128, tsl]
                        )
                sqt = sm.tile([128, NQ], BF16, name=f"sqt{t}", tag="sqt", bufs=4)
                sqt_tiles.append(sqt)
                nc.vector.tensor_mul(sqt[:], o_bf[:, tsl], o_bf[:, tsl])
                # LN0 partial sums for block t (PE filler, col-tiled rows 0/64)
                for c in range(2):
                    csl = slice(t * NQ + c * 512, t * NQ + c * 512 + 512)
                    mm(sts0[c][0:1, :], sum_lhs, o_bf[:, csl],
                       start=(t == 0), stop=(t == 3))
                    mm(sts0[c][64:65, :], sum_lhs,
                       sqt[:, c * 512 : c * 512 + 512],
                       start=(t == 0), stop=(t == 3))

            # ---- pipelined main loop -----------------------------------------
            # window t: scores(t)/exps(t) paced by ACT, with A@V(t-1) in
            # 5-mm chunks + proj(t+1) as PE filler between score tiles.
            # Startup order matches DMA arrival: kt,wk -> k-proj; wv -> v-proj;
            # qt,wq -> q-proj.
            proj_k(0)
            for vt in range(kt_tiles):
                vproj(vt)
            proj_q(0)
            for t in range(4):
                at_tiles[t] = {
                    sub: [
                        ap.tile([128, NQ], BF16, name=f"at{t}_{i}_{sub}",
                                tag="at", bufs=20)
                        for i in range(kt_tiles)
                    ]
                    for sub in range(2)
                }
                # filler schedule: one item after each scores tile
                filler = []
                if t == 0:
                    filler = [lambda: proj_k(1), lambda: proj_q(1)]
                else:
                    filler = [
                        lambda: av_chunk(t - 1, 0, 0),
                        lambda: av_chunk(t - 1, 0, 1),
                        lambda: av_chunk(t - 1, 1, 0),
                        lambda: av_chunk(t - 1, 1, 1),
                        lambda: norm_lnexp(t - 1),
                        lambda: norm_apply(t - 1),
                    ]
                    if t < 3:
                        filler.insert(2, lambda: proj_k(t + 1))
                        filler.insert(5, lambda: proj_q(t + 1))
                fi = 0
                for i in range(kt_tiles):
                    scores_i(t, i)
                    # ~1-2 filler items between score tiles
                    take = 2 if i >= kt_tiles - 2 else 1
                    for _ in range(take):
                        if fi < len(filler):
                            filler[fi]()
                            fi += 1
                while fi < len(filler):
                    filler[fi]()
                    fi += 1
            for sub in range(2):
                for c in range(2):
                    av_chunk(3, sub, c)
            norm_lnexp(3)
            norm_apply(3)

            # ---- LN stats + broadcast, one c-chunk at a time.  Chain is
            # latency-critical: square on DVE, rr broadcast emitted as early
            # as possible so downstream applies start sooner. ------------------
            def ln_stats_c(sts, c, tag):
                """Returns (rr_c, rm_c): [128,512] bf16 broadcasts for chunk c."""
                var = sm.tile([1, 512], F32, name=f"var{tag}{c}", tag="var",
                              bufs=2)
                mu = sm.tile([1, 512], F32, name=f"mu{tag}{c}", tag="mu", bufs=2)
                nc.scalar.activation(var[:], sts[c][0:1, :], AF.Square)
                nc.vector.tensor_sub(var[:], sts[c][64:65, :], var[:])
                nc.vector.tensor_copy(mu[:], sts[c][0:1, :])
                rstd = sm.tile([1, 512], BF16, name=f"rs{tag}{c}", tag="rs",
                               bufs=2)
                murm = sm.tile([1, 512], BF16, name=f"mm{tag}{c}", tag="mm2",
                               bufs=2)
                nc.scalar.activation(var[:], var[:], AF.Ln,
                                     bias=eps_sb[0:1, 0:1])
                nc.scalar.activation(rstd[:], var[:], AF.Exp, scale=-0.5)
                rr = sm.tile([128, 512], BF16, name=f"rrb{tag}{c}", tag="rrb",
                             bufs=2)
                rm = sm.tile([128, 512], BF16, name=f"rmb{tag}{c}", tag="rmb",
                             bufs=2)
                rr_ps = po_tile(f"rr{tag}{c}")
                mm(rr_ps[:, :], ones_row, rstd[:], start=True, stop=True)
                nc.scalar.copy(rr[:], rr_ps[:, :])
                nc.vector.tensor_mul(murm[:], mu[:], rstd[:])
                rm_ps = po_tile(f"rm{tag}{c}")
                mm(rm_ps[:, :], ones_row, murm[:], start=True, stop=True)
                nc.scalar.copy(rm[:], rm_ps[:, :])
                return rr, rm

            # ---- LN0 apply + FC + LN1, software-pipelined across c-chunks ----
            ot0 = ap.tile([128, 4 * NQ], BF16, name="ot0")
            o1 = ap.tile([128, 4 * NQ], BF16, name="o1")
            otout = ap.tile([128, 4 * NQ], BF16, name="otout")
            sq1_tiles = [
                sm.tile([128, NQ], BF16, name=f"sq1t{ot}", tag="sqt", bufs=4)
                for ot in range(4)
            ]
            sts1 = [spb_tile(f"stl1{c}") for c in range(2)]
            oq = 0

            def ln0_apply(c, rr0, rm0):
                for t in range(4):
                    sl = slice(t * NQ + c * 512, t * NQ + c * 512 + 512)
                    nc.vector.tensor_mul(ot0[:, sl], o_bf[:, sl], rr0[:])
                    nc.vector.tensor_sub(ot0[:, sl], ot0[:, sl], rm0[:])
                    if affine:
                        nc.vector.tensor_scalar(
                            ot0[:, sl], ot0[:, sl],
                            gb_sb[:, 0 + t : 0 + t + 1],
                            gb_sb[:, 4 + t : 4 + t + 1],
                            mult, add,
                        )

            def fc_ot(c, ot):
                ps_f = sx_tile(f"psf{ot}_{c}")
                for ft in range(4):
                    mm(
                        ps_f[:, 0:512],
                        wo_sb[:, ft * D + ot * 128 : ft * D + (ot + 1) * 128],
                        ot0[:, ft * NQ + c * 512 : ft * NQ + c * 512 + 512],
                        start=(ft == 0),
                        stop=(ft == 3),
                    )
                csl = slice(ot * NQ + c * 512, ot * NQ + c * 512 + 512)
                rl = sm.tile([128, 512], BF16, name=f"rl{ot}{c}", tag="rl",
                             bufs=2)
                nc.vector.tensor_scalar(
                    rl[:], ps_f[:, 0:512], bo_sb[:, ot : ot + 1], 0.0,
                    add, amax,
                )
                nc.vector.tensor_add(o1[:, csl], ot0[:, csl], rl[:])
                sq1t = sq1_tiles[ot]
                nc.scalar.activation(
                    sq1t[:, c * 512 : c * 512 + 512], o1[:, csl], AF.Square
                )
                mm(sts1[c][0:1, :], sum_lhs, o1[:, csl],
                   start=(ot == 0), stop=(ot == 3))
                mm(sts1[c][64:65, :], sum_lhs,
                   sq1t[:, c * 512 : c * 512 + 512],
                   start=(ot == 0), stop=(ot == 3))

            def ln1_apply(c, t, rr1, rm1):
                nonlocal oq
                sl = slice(t * NQ + c * 512, t * NQ + c * 512 + 512)
                tmp = sm.tile([128, 512], BF16, name=f"tmp{t}{c}", tag="rl",
                              bufs=2)
                nc.vector.tensor_mul(tmp[:], o1[:, sl], rr1[:])
                if affine:
                    nc.vector.tensor_sub(tmp[:], tmp[:], rm1[:])
                    nc.vector.tensor_scalar(
                        otout[:, sl], tmp[:],
                        gb_sb[:, 8 + t : 8 + t + 1],
                        gb_sb[:, 12 + t : 12 + t + 1],
                        mult, add,
                    )
                else:
                    nc.vector.tensor_sub(otout[:, sl], tmp[:], rm1[:])
                QS[oq % 3].dma_start(
                    out_d[t * 128 : (t + 1) * 128, c * 512 : c * 512 + 512],
                    otout[:, sl],
                )
                oq += 1

            # pipeline: stats_c0 | apply_c0 | stats_c1 early | FC_c0 |
            # apply_c1 | ln1stats_c0 | FC_c1 interleaved with ln1_apply_c0 |
            # ln1stats_c1 | ln1_apply_c1
            r0c0 = ln_stats_c(sts0, 0, "l0")
            ln0_apply(0, *r0c0)
            r0c1 = ln_stats_c(sts0, 1, "l0")
            for ot in range(4):
                fc_ot(0, ot)
            ln0_apply(1, *r0c1)
            r1c0 = ln_stats_c(sts1, 0, "l1")
            for ot in range(4):
                fc_ot(1, ot)
                ln1_apply(0, ot, *r1c0)
            r1c1 = ln_stats_c(sts1, 1, "l1")
            for t in range(4):
                ln1_apply(1, t, *r1c1)

    _split_multi_waits(nc)
    return nc


_nc_cache = {}


def _get_nc(kt_tiles=5, affine=False):
    key = (kt_tiles, affine)
    if key not in _nc_cache:
        _nc_cache[key] = build_nc(kt_tiles, affine)
    return _nc_cache[key]


def _kt_tiles_for(mask):
    n = int(max(int((mask[b] != 0).sum()) for b in range(mask.shape[0])))
    return max(1, (n + 127) // 128)


def _is_affine(g0, b0, g1, b1):
    return not (
        np.all(np.asarray(g0) == 1.0)
        and np.all(np.asarray(b0) == 0.0)
        and np.all(np.asarray(g1) == 1.0)
        and np.all(np.asarray(b1) == 0.0)
    )


def prep_inputs(Q, K, mask, Wq, bq, Wk, bk, Wv, bv, Wo, bo, g0, b0, g1, b1,
                kt_tiles=None):
    f32, bf = np.float32, ml_dtypes.bfloat16
    if kt_tiles is None:
        kt_tiles = _kt_tiles_for(np.asarray(mask))
    nkp = kt_tiles * 128

    def percol(v):  # [512] feature vector -> [128, 4] per-partition layout
        return np.ascontiguousarray(np.asarray(v, f32).reshape(4, 128).T)

    wv_h = np.ascontiguousarray(
        np.vstack([np.asarray(Wv, f32), np.asarray(bv, f32)[None, :]])
    ).astype(bf)
    cr = np.zeros((33, 128), f32)
    cr[0, :] = 1.0
    cr[32, :] = 1.0
    cr = cr.astype(bf)
    gb = np.concatenate(
        [percol(g0), percol(b0), percol(g1), percol(b1)], axis=1
    ).astype(f32)
    cn = np.full((128, 1), 1.0 / D, f32).astype(bf)
    wq_h = np.ascontiguousarray(np.asarray(Wq, f32)).astype(bf)
    wk_h = np.ascontiguousarray(np.asarray(Wk, f32)).astype(bf)
    wo_h = np.ascontiguousarray(np.asarray(Wo, f32)).astype(bf)

    in_maps = []
    for b in range(B):
        qt = np.ascontiguousarray(np.asarray(Q[b], f32).T).astype(bf)
        idx = np.nonzero(np.asarray(mask)[b] != 0)[0]
        kc = np.zeros((nkp, D), f32)
        kc[: len(idx)] = np.asarray(K[b], f32)[idx]
        indrow = np.zeros((1, nkp), f32)
        indrow[0, : len(idx)] = 1.0
        kt = np.ascontiguousarray(np.vstack([kc.T, indrow])).astype(bf)
        ind = np.ascontiguousarray(indrow.reshape(kt_tiles, 128).T).astype(bf)
        in_maps.append(
            {
                "qt": qt,
                "kt": kt,
                "wq": wq_h,
                "wk": wk_h,
                "wv": wv_h,
                "wo": wo_h,
                "bq": percol(bq),
                "bk": percol(bk),
                "bo": percol(bo),
                "ind": ind,
                "cr": cr,
                "gb": gb,
                "cn": cn,
            }
        )
    return in_maps


def kernel(Q, K, mask, Wq, bq, Wk, bk, Wv, bv, Wo, bo, g0, b0, g1, b1):
    mask = np.asarray(mask)
    kt_tiles = _kt_tiles_for(mask)
    affine = _is_affine(g0, b0, g1, b1)
    nc = _get_nc(kt_tiles, affine)
    in_maps = prep_inputs(
        Q, K, mask, Wq, bq, Wk, bk, Wv, bv, Wo, bo, g0, b0, g1, b1, kt_tiles
    )
    res = run_bass_kernel_spmd(nc, in_maps, list(range(N_CORES)))
    out = np.stack(
        [np.ascontiguousarray(res.results[i]["out"].T) for i in range(N_CORES)]
    )
    return out.astype(np.float32)
